# revision 25
# baseline (speedup 1.0000x reference)
"""Trainium2 Bass kernel for nn_CrossLayer (DCN-style cross stack).

Reference semantics (B=16384, D=1024, L=8):
    out_0 = x
    s_i = einsum('bd,d->b', out_i, W[i])
    out_{i+1} = x * s_i[:, None] + b[i] + x

Algebraic collapse: out = x * rho[:, None] + b[L-1] with
    rho_1 = u_0 + 1,   rho_{l+1} = rho_l * u_l + c_l
    u_l[r] = <x[r, :], W[l]>          (U = x @ W.T, [B, L])
    c_l = <b[l-1], W[l]> + 1          (weights-only scalars)

fp16 data path (correctness gate is scale-relative ~2e-2; fp16
end-to-end measures ~9e-4 scale-relative absmax). Host converts x to
fp16 and pre-transposes it per 512-row block, so the device never
transposes x: U comes from W-stationary matmuls over the transposed
layout. Per block: the 7-step scan runs as fused scalar_tensor_tensor
ops on [128, NT] tiles (shifted variable: sig_{i+1} = (sig_i +
c_{i-1}) * u_{i+1}; rho = sig_7 + c_6 folded into the fp16 narrowing),
rho is replicated across partitions with one K=1 ones-matmul, and
yT = xT * rhoRep + b7 runs as broadcast tensor_tensor multiplies
(fp16 2x DVE mode) plus per-chunk bias adds split between the DVE and
the scalar engine. The final 512 rows are processed as two 256-row
sub-blocks so the exposed tail chain is short. yT leaves in fp16; the
host transposes back and widens to f32.

HBM floor per core: 8.4MB @ 360GB/s ~ 23.4us (vs 16.8MB/47us f32).

DRAM layouts keep (chunk, row) contiguous per partition row (4KB DMA
lines): xt[s*128 + p, c*RSG + r] = x[s*RSG + r, c*128 + p].
"""

import numpy as np

import concourse.bacc as bacc
import concourse.tile as tile
from concourse import mybir
from concourse.bass_utils import run_bass_kernel_spmd
from concourse.masks import make_identity

N_CORES = 8
B, D, L = 16384, 1024, 8
RPC = B // N_CORES          # rows per core (2048)
NCH = D // 128              # 128-wide d chunks (8)
NSG = 4                     # 512-row DRAM blocks per core
RSG = RPC // NSG            # rows per DRAM block (512)
N_WARM = 8                  # fp16 warmup matmuls to start the PE ramp
DVE_TS = (0, 2, 4, 6)       # chunks whose +b7 runs on DVE (rest: scalar)
# processing blocks: (dram_block, col0, rows) — tail split in two
BLOCKS = [(0, 0, 512), (1, 0, 512), (2, 0, 512), (3, 0, 256), (3, 256, 256)]

LAST_RESULTS = None


def _build(cvals):
    """Trace + compile the per-core program. cvals = [c_1..c_{L-1}] (f64->f32)."""
    nc = bacc.Bacc("TRN2", target_bir_lowering=False, debug=False)
    f32 = mybir.dt.float32
    f16 = mybir.dt.float16
    mult = mybir.AluOpType.mult
    add = mybir.AluOpType.add

    xt_d = nc.dram_tensor("xt", [NSG * 128, NCH * RSG], f16, kind="ExternalInput")
    wt_d = nc.dram_tensor("wt", [128, NCH * L], f16, kind="ExternalInput")
    b7_d = nc.dram_tensor("b7c", [128, NCH], f32, kind="ExternalInput")
    yt_d = nc.dram_tensor("yt", [NSG * 128, NCH * RSG], f16, kind="ExternalOutput")

    xt_vw = xt_d.ap().rearrange("(s p) (c r) -> s p c r", p=128, c=NCH)
    yt_vw = yt_d.ap().rearrange("(s p) (c r) -> s p c r", p=128, c=NCH)

    with tile.TileContext(nc) as tc:
        with (
            # PSUM pools, creation order fixes bank layout (2KB bank each buf):
            #  pU x2 -> banks 0,1 | pB x2 -> banks 2,3 (warmups share psB)
            #  pT x2 -> banks 4,5 | pR x2 -> bank 6
            tc.tile_pool(name="pU", bufs=2, space="PSUM") as pU,
            tc.tile_pool(name="pB", bufs=2, space="PSUM") as pB,
            tc.tile_pool(name="pT", bufs=2, space="PSUM") as pT,
            tc.tile_pool(name="pR", bufs=2, space="PSUM") as pR,
            tc.tile_pool(name="const", bufs=1) as cpool,
            tc.tile_pool(name="xp", bufs=6) as xpool,
            tc.tile_pool(name="yp", bufs=3) as ypool,
            tc.tile_pool(name="sm", bufs=2) as spool,
        ):
            # --- tiny const DMAs on the scalar HWDGE queue so the sync queue
            # belongs to the bulk x stream from instruction zero ---
            wt_sb = cpool.tile([128, NCH, L], f16)
            nc.scalar.dma_start(out=wt_sb[:], in_=wt_d.ap().rearrange("p (c l) -> p c l", l=L))
            b7_sb = cpool.tile([128, NCH], f32)
            nc.scalar.dma_start(out=b7_sb[:], in_=b7_d[:, :])

            # --- all x data on the wire, half-block granularity so the first
            # U matmuls start as soon as chunks 0-3 of block 0 land ---
            xg = []
            for bi, (s, col0, rows) in enumerate(BLOCKS):
                halves = []
                for h in range(2):
                    xs = xpool.tile(
                        [128, NCH // 2, rows], f16, tag=f"x{h}", name=f"xg{bi}_{h}",
                        padded_shape=[128, NCH // 2, RSG],
                    )
                    nc.sync.dma_start(
                        out=xs[:],
                        in_=xt_vw[s][:, 4 * h : 4 * h + 4, col0 : col0 + rows],
                    )
                    halves.append(xs)
                xg.append(halves)

            # --- warmup: fp16 matmuls to start the PE power ramp ---
            dummy = cpool.tile([128, 512], f16)
            nc.gpsimd.memset(dummy[:], 0.0)
            for i in range(N_WARM):
                pw = pB.tile([128, 512], f32, tag="psB", name=f"pw{i}")
                nc.tensor.matmul(pw[:], dummy[:, 0:128], dummy[:], start=True, stop=True)

            # --- constants ---
            ident = cpool.tile([128, 128], f32)
            make_identity(nc, ident[:])
            ones = cpool.tile([1, 128], f16)
            nc.gpsimd.memset(ones[:], 1.0)
            c6b = cpool.tile([128, 1], f32)
            nc.gpsimd.memset(c6b[:], float(cvals[L - 2]))

            def emit_U(bi):
                """U^T for block bi: psU[l, r] = sum_c <W_c[:, l], xT_c[:, r]>."""
                rows = BLOCKS[bi][2]
                psU = pU.tile([L, rows], f32, tag="psU", name=f"psU{bi}",
                              padded_shape=[128, RSG])
                for c in range(NCH):
                    nc.tensor.matmul(
                        psU[:], wt_sb[:, c, :], xg[bi][c // 4][:, c % 4, :],
                        start=(c == 0), stop=(c == NCH - 1),
                    )
                return psU

            def emit_chain_a(bi, psU):
                """Chain front half: psU -> SBUF -> psR (row orientation)."""
                rows = BLOCKS[bi][2]
                nt = rows // 128
                ut = spool.tile([L, rows], f32, tag="ut", name=f"ut{bi}",
                                padded_shape=[128, RSG])
                nc.scalar.copy(ut[:], psU[:])

                # back to row-partition orientation: psR[p, t, l] (PSUM)
                psR = pR.tile([128, nt, L], f32, tag="psR", name=f"psR{bi}",
                              padded_shape=[128, RSG // 128, L])
                for t in range(nt):
                    nc.tensor.transpose(
                        psR[:, t, :], ut[:, 128 * t : 128 * (t + 1)], ident[0:L, 0:L]
                    )
                return psR

            def emit_chain_b(bi, psR):
                """Chain back half: scan -> rho -> rhoR (replicated, fp16)."""
                rows = BLOCKS[bi][2]
                nt = rows // 128
                # scan (DVE reads U straight out of PSUM)
                sig = [
                    spool.tile([128, nt], f32, tag=f"sig{i}", name=f"sig{bi}_{i}",
                               padded_shape=[128, RSG // 128])
                    for i in range(2)
                ]
                nc.vector.tensor_scalar_add(sig[0][:], psR[:, :, 0], 1.0)
                for i in range(L - 1):
                    d_i = 0.0 if i == 0 else cvals[i - 1]
                    nc.vector.scalar_tensor_tensor(
                        sig[(i + 1) % 2][:], sig[i % 2][:], d_i,
                        psR[:, :, i + 1], add, mult,
                    )
                rho_f = sig[(L - 1) % 2]

                # rho columns -> partition 0: psT[0, t*128+r] = rho[tile t, r]
                psT = pT.tile([1, nt, 128], f32, tag="psT", name=f"psT{bi}",
                              padded_shape=[128, RSG // 128, 128])
                for t in range(nt):
                    nc.tensor.transpose(psT[0:1, t, :], rho_f[:, t : t + 1], ident[:])
                # +c_6 fused into the fp16 narrowing copy (scalar engine)
                rhoT = spool.tile([1, rows], f16, tag="rhoT", name=f"rhoT{bi}",
                                  padded_shape=[128, RSG])
                nc.scalar.add(rhoT[:], psT[:].rearrange("p t r -> p (t r)"), c6b[0:1, :])

                # one K=1 matmul replicates rho across partitions
                psB = pB.tile([128, rows], f32, tag="psB", name=f"psB{bi}",
                              padded_shape=[128, RSG])
                nc.tensor.matmul(psB[:], ones[:], rhoT[:], start=True, stop=True)
                rhoR = spool.tile([128, 1, rows], f16, tag="rhoR", name=f"rhoR{bi}",
                                  padded_shape=[128, 1, RSG])
                nc.scalar.copy(rhoR[:].rearrange("p o r -> p (o r)"), psB[:])
                return rhoR

            def emit_y(bi, rhoR):
                """yT = xT * rhoRep + b7; stream out in two halves."""
                s, col0, rows = BLOCKS[bi]
                ys = ypool.tile([128, NCH, rows], f16, tag="yg", name=f"yg{bi}",
                                padded_shape=[128, NCH, RSG])
                rep = rhoR[:].broadcast_to([128, 4, rows])
                for half in range(2):
                    h0 = 4 * half
                    nc.vector.tensor_mul(ys[:, h0 : h0 + 4, :], xg[bi][half][:], rep)
                    for c in range(h0, h0 + 4):
                        if c in DVE_TS:
                            nc.vector.tensor_scalar_add(
                                ys[:, c, :], ys[:, c, :], b7_sb[:, c : c + 1]
                            )
                        else:
                            nc.scalar.add(
                                ys[:, c, :], ys[:, c, :], b7_sb[:, c : c + 1]
                            )
                    nc.gpsimd.dma_start(
                        out=yt_vw[s][:, h0 : h0 + 4, col0 : col0 + rows],
                        in_=ys[:, h0 : h0 + 4, :],
                    )

            # software pipeline: chain front halves go right after their U so
            # the PE reaches tr(b) early; back halves go after the NEXT U so
            # the scan-wait never bubbles the PE; y trails by one block
            NB = len(BLOCKS)
            psU0 = emit_U(0)
            psU1 = emit_U(1)
            psR0 = emit_chain_a(0, psU0)
            psU2 = emit_U(2)
            rho0 = emit_chain_b(0, psR0)
            psR1 = emit_chain_a(1, psU1)
            emit_y(0, rho0)
            psU3 = emit_U(3)
            rho1 = emit_chain_b(1, psR1)
            psR2 = emit_chain_a(2, psU2)
            emit_y(1, rho1)
            psU4 = emit_U(4)
            rho2 = emit_chain_b(2, psR2)
            psR3 = emit_chain_a(3, psU3)
            emit_y(2, rho2)
            rho3 = emit_chain_b(3, psR3)
            psR4 = emit_chain_a(4, psU4)
            emit_y(3, rho3)
            rho4 = emit_chain_b(4, psR4)
            emit_y(4, rho4)

    nc.compile()
    return nc


def kernel(x, W, b):
    global LAST_RESULTS
    x = np.asarray(x)
    W = np.asarray(W)
    b = np.asarray(b)
    assert x.shape == (B, D) and W.shape == (L, D) and b.shape == (L, D)

    cvals = [float(np.dot(b[l - 1].astype(np.float64), W[l].astype(np.float64)) + 1.0)
             for l in range(1, L)]

    # weights: wt[p, c*L + l] = W[l, c*128 + p]
    wt = W.T.reshape(NCH, 128, L).transpose(1, 0, 2).reshape(128, NCH * L)
    wt = np.ascontiguousarray(wt, dtype=np.float16)
    # b7c[p, c] = b[L-1, c*128 + p]
    b7c = np.ascontiguousarray(b[L - 1].reshape(NCH, 128).T, dtype=np.float32)

    # x: fp16, blocked transpose with (chunk, row) contiguous per partition:
    # xt[s*128+p, c*RSG+r] = x[s*RSG+r, c*128+p]
    x16 = x.astype(np.float16)
    shards = []
    for i in range(N_CORES):
        xc = x16[i * RPC : (i + 1) * RPC]                       # [RPC, D]
        xt = xc.reshape(NSG, RSG, NCH, 128).transpose(0, 3, 2, 1)
        shards.append(np.ascontiguousarray(xt).reshape(NSG * 128, NCH * RSG))

    nc = _build(cvals)

    in_maps = [{"xt": s, "wt": wt, "b7c": b7c} for s in shards]
    res = run_bass_kernel_spmd(nc, in_maps, core_ids=list(range(N_CORES)))
    LAST_RESULTS = res

    out = np.empty((B, D), dtype=np.float32)
    for i in range(N_CORES):
        yt = res.results[i]["yt"].reshape(NSG, 128, NCH, RSG)
        out[i * RPC : (i + 1) * RPC] = (
            yt.transpose(0, 3, 2, 1).reshape(RPC, D).astype(np.float32)
        )
    return out


# revision 27
# speedup vs baseline: 1.1520x; 1.1520x over previous
"""Trainium2 Bass kernel for nn_CrossLayer (DCN-style cross stack).

Reference semantics (B=16384, D=1024, L=8):
    out_0 = x
    s_i = einsum('bd,d->b', out_i, W[i])
    out_{i+1} = x * s_i[:, None] + b[i] + x

Algebraic collapse: out = x * rho[:, None] + b[L-1] with
    rho_1 = u_0 + 1,   rho_{l+1} = rho_l * u_l + c_l
    u_l[r] = <x[r, :], W[l]>          (U = x @ W.T, [B, L])
    c_l = <b[l-1], W[l]> + 1          (weights-only scalars)

fp16 data path (correctness gate is scale-relative ~2e-2; fp16
end-to-end measures ~9e-4 scale-relative absmax). Host converts x to
fp16 and pre-transposes it per 512-row block, so the device never
transposes x: U comes from W-stationary matmuls over the transposed
layout. Per block: the 7-step scan runs as fused scalar_tensor_tensor
ops on [128, NT] tiles (shifted variable: sig_{i+1} = (sig_i +
c_{i-1}) * u_{i+1}; rho = sig_7 + c_6 folded into the fp16 narrowing),
rho is replicated across partitions with one K=1 ones-matmul, and
yT = xT * rhoRep + b7 runs as broadcast tensor_tensor multiplies
(fp16 2x DVE mode) plus per-chunk bias adds split between the DVE and
the scalar engine. The final 512 rows are processed as two 256-row
sub-blocks so the exposed tail chain is short. yT leaves in fp16; the
host transposes back and widens to f32.

HBM floor per core: 8.4MB @ 360GB/s ~ 23.4us (vs 16.8MB/47us f32).

DRAM layouts keep (chunk, row) contiguous per partition row (4KB DMA
lines): xt[s*128 + p, c*RSG + r] = x[s*RSG + r, c*128 + p].
"""

import numpy as np

import concourse.bacc as bacc
import concourse.tile as tile
from concourse import mybir
from concourse.bass_utils import run_bass_kernel_spmd
from concourse.masks import make_identity

N_CORES = 8
B, D, L = 16384, 1024, 8
RPC = B // N_CORES          # rows per core (2048)
NCH = D // 128              # 128-wide d chunks (8)
NSG = 4                     # 512-row DRAM blocks per core
RSG = RPC // NSG            # rows per DRAM block (512)
N_WARM = 8                  # fp16 warmup matmuls to start the PE ramp
DVE_TS = (0, 2, 4, 6)       # chunks whose +b7 runs on DVE (rest: scalar)
# processing blocks: (dram_block, col0, rows) — tail split in two
BLOCKS = [(0, 0, 512), (1, 0, 512), (2, 0, 512), (3, 0, 256), (3, 256, 256)]

LAST_RESULTS = None


def _build(cvals):
    """Trace + compile the per-core program. cvals = [c_1..c_{L-1}] (f64->f32)."""
    nc = bacc.Bacc("TRN2", target_bir_lowering=False, debug=False)
    f32 = mybir.dt.float32
    f16 = mybir.dt.float16
    mult = mybir.AluOpType.mult
    add = mybir.AluOpType.add

    xt_d = nc.dram_tensor("xt", [NSG * 128, NCH * RSG], f16, kind="ExternalInput")
    wt_d = nc.dram_tensor("wt", [128, NCH * L], f16, kind="ExternalInput")
    b7_d = nc.dram_tensor("b7c", [128, NCH], f32, kind="ExternalInput")
    yt_d = nc.dram_tensor("yt", [NSG * 128, NCH * RSG], f16, kind="ExternalOutput")

    xt_vw = xt_d.ap().rearrange("(s p) (c r) -> s p c r", p=128, c=NCH)
    yt_vw = yt_d.ap().rearrange("(s p) (c r) -> s p c r", p=128, c=NCH)

    with tile.TileContext(nc) as tc:
        with (
            # PSUM pools, creation order fixes bank layout (2KB bank each buf):
            #  pU x2 -> banks 0,1 | pB x2 -> banks 2,3 (warmups share psB)
            #  pT x2 -> banks 4,5 | pR x2 -> bank 6
            tc.tile_pool(name="pU", bufs=2, space="PSUM") as pU,
            tc.tile_pool(name="pB", bufs=2, space="PSUM") as pB,
            tc.tile_pool(name="pT", bufs=2, space="PSUM") as pT,
            tc.tile_pool(name="pR", bufs=2, space="PSUM") as pR,
            tc.tile_pool(name="const", bufs=1) as cpool,
            tc.tile_pool(name="xp", bufs=6) as xpool,
            tc.tile_pool(name="yp", bufs=3) as ypool,
            tc.tile_pool(name="sm", bufs=2) as spool,
        ):
            # --- tiny const DMAs on the scalar HWDGE queue so the sync queue
            # belongs to the bulk x stream from instruction zero ---
            wt_sb = cpool.tile([128, NCH, L], f16)
            nc.scalar.dma_start(out=wt_sb[:], in_=wt_d.ap().rearrange("p (c l) -> p c l", l=L))
            b7_sb = cpool.tile([128, NCH], f32)
            nc.scalar.dma_start(out=b7_sb[:], in_=b7_d[:, :])

            # --- all x data on the wire, half-block granularity so the first
            # U matmuls start as soon as chunks 0-3 of block 0 land ---
            xg = []
            for bi, (s, col0, rows) in enumerate(BLOCKS):
                halves = []
                for h in range(2):
                    xs = xpool.tile(
                        [128, NCH // 2, rows], f16, tag=f"x{h}", name=f"xg{bi}_{h}",
                        padded_shape=[128, NCH // 2, RSG],
                    )
                    nc.sync.dma_start(
                        out=xs[:],
                        in_=xt_vw[s][:, 4 * h : 4 * h + 4, col0 : col0 + rows],
                    )
                    halves.append(xs)
                xg.append(halves)

            # --- warmup: fp16 matmuls to start the PE power ramp ---
            dummy = cpool.tile([128, 512], f16)
            nc.gpsimd.memset(dummy[:], 0.0)
            for i in range(N_WARM):
                pw = pB.tile([128, 512], f32, tag="psB", name=f"pw{i}")
                nc.tensor.matmul(pw[:], dummy[:, 0:128], dummy[:], start=True, stop=True)

            # --- constants ---
            ident = cpool.tile([128, 128], f32)
            make_identity(nc, ident[:])
            ones = cpool.tile([1, 128], f16)
            nc.gpsimd.memset(ones[:], 1.0)
            c6b = cpool.tile([128, 1], f32)
            nc.gpsimd.memset(c6b[:], float(cvals[L - 2]))

            def emit_U(bi):
                """U^T for block bi: psU[l, r] = sum_c <W_c[:, l], xT_c[:, r]>."""
                rows = BLOCKS[bi][2]
                psU = pU.tile([L, rows], f32, tag="psU", name=f"psU{bi}",
                              padded_shape=[128, RSG])
                for c in range(NCH):
                    nc.tensor.matmul(
                        psU[:], wt_sb[:, c, :], xg[bi][c // 4][:, c % 4, :],
                        start=(c == 0), stop=(c == NCH - 1),
                    )
                return psU

            def emit_chain_a(bi, psU):
                """Chain front half: psU -> SBUF -> psR (row orientation)."""
                rows = BLOCKS[bi][2]
                nt = rows // 128
                ut = spool.tile([L, rows], f32, tag="ut", name=f"ut{bi}",
                                padded_shape=[128, RSG])
                nc.scalar.copy(ut[:], psU[:])

                # back to row-partition orientation: psR[p, t, l] (PSUM)
                psR = pR.tile([128, nt, L], f32, tag="psR", name=f"psR{bi}",
                              padded_shape=[128, RSG // 128, L])
                for t in range(nt):
                    nc.tensor.transpose(
                        psR[:, t, :], ut[:, 128 * t : 128 * (t + 1)], ident[0:L, 0:L]
                    )
                return psR

            def emit_chain_b(bi, psR):
                """Chain middle: scan (DVE, reads PSUM) + rho -> partition 0."""
                rows = BLOCKS[bi][2]
                nt = rows // 128
                sig = [
                    spool.tile([128, nt], f32, tag=f"sig{i}", name=f"sig{bi}_{i}",
                               padded_shape=[128, RSG // 128])
                    for i in range(2)
                ]
                nc.vector.tensor_scalar_add(sig[0][:], psR[:, :, 0], 1.0)
                for i in range(L - 1):
                    d_i = 0.0 if i == 0 else cvals[i - 1]
                    nc.vector.scalar_tensor_tensor(
                        sig[(i + 1) % 2][:], sig[i % 2][:], d_i,
                        psR[:, :, i + 1], add, mult,
                    )
                rho_f = sig[(L - 1) % 2]

                # rho columns -> partition 0: psT[0, t*128+r] = rho[tile t, r]
                psT = pT.tile([1, nt, 128], f32, tag="psT", name=f"psT{bi}",
                              padded_shape=[128, RSG // 128, 128])
                for t in range(nt):
                    nc.tensor.transpose(psT[0:1, t, :], rho_f[:, t : t + 1], ident[:])
                return psT

            def emit_chain_c(bi, psT):
                """Chain tail: fp16 narrowing (+c_6) and partition replication."""
                rows = BLOCKS[bi][2]
                nt = rows // 128
                rhoT = spool.tile([1, rows], f16, tag="rhoT", name=f"rhoT{bi}",
                                  padded_shape=[128, RSG])
                nc.scalar.add(
                    rhoT[:], psT[0:1, 0:nt, :].rearrange("p t r -> p (t r)"),
                    c6b[0:1, :],
                )
                # one K=1 matmul replicates rho across partitions
                psB = pB.tile([128, rows], f32, tag="psB", name=f"psB{bi}",
                              padded_shape=[128, RSG])
                nc.tensor.matmul(psB[:], ones[:], rhoT[:], start=True, stop=True)
                rhoR = spool.tile([128, 1, rows], f16, tag="rhoR", name=f"rhoR{bi}",
                                  padded_shape=[128, 1, RSG])
                nc.scalar.copy(rhoR[:].rearrange("p o r -> p (o r)"), psB[:])
                return rhoR

            def emit_y(bi, rhoR):
                """yT = xT * rhoRep + b7; stream out in two halves."""
                s, col0, rows = BLOCKS[bi]
                ys = ypool.tile([128, NCH, rows], f16, tag="yg", name=f"yg{bi}",
                                padded_shape=[128, NCH, RSG])
                rep = rhoR[:].broadcast_to([128, 4, rows])
                for half in range(2):
                    h0 = 4 * half
                    nc.vector.tensor_mul(ys[:, h0 : h0 + 4, :], xg[bi][half][:], rep)
                    for c in range(h0, h0 + 4):
                        if c in DVE_TS:
                            nc.vector.tensor_scalar_add(
                                ys[:, c, :], ys[:, c, :], b7_sb[:, c : c + 1]
                            )
                        else:
                            nc.scalar.add(
                                ys[:, c, :], ys[:, c, :], b7_sb[:, c : c + 1]
                            )
                    nc.gpsimd.dma_start(
                        out=yt_vw[s][:, h0 : h0 + 4, col0 : col0 + rows],
                        in_=ys[:, h0 : h0 + 4, :],
                    )

            # software pipeline: chain front halves go right after their U so
            # the PE reaches tr(b) early; back halves go after the NEXT U so
            # the scan-wait never bubbles the PE; y trails by one block
            psU0 = emit_U(0)
            psU1 = emit_U(1)
            psR0 = emit_chain_a(0, psU0)
            psU2 = emit_U(2)
            psT0 = emit_chain_b(0, psR0)
            psR1 = emit_chain_a(1, psU1)
            rho0 = emit_chain_c(0, psT0)
            emit_y(0, rho0)
            psU3 = emit_U(3)
            psT1 = emit_chain_b(1, psR1)
            psR2 = emit_chain_a(2, psU2)
            rho1 = emit_chain_c(1, psT1)
            emit_y(1, rho1)
            psU4 = emit_U(4)
            psT2 = emit_chain_b(2, psR2)
            psR3 = emit_chain_a(3, psU3)
            rho2 = emit_chain_c(2, psT2)
            emit_y(2, rho2)
            psT3 = emit_chain_b(3, psR3)
            psR4 = emit_chain_a(4, psU4)
            rho3 = emit_chain_c(3, psT3)
            emit_y(3, rho3)
            psT4 = emit_chain_b(4, psR4)
            rho4 = emit_chain_c(4, psT4)
            emit_y(4, rho4)

    nc.compile()
    return nc


def kernel(x, W, b):
    global LAST_RESULTS
    x = np.asarray(x)
    W = np.asarray(W)
    b = np.asarray(b)
    assert x.shape == (B, D) and W.shape == (L, D) and b.shape == (L, D)

    cvals = [float(np.dot(b[l - 1].astype(np.float64), W[l].astype(np.float64)) + 1.0)
             for l in range(1, L)]

    # weights: wt[p, c*L + l] = W[l, c*128 + p]
    wt = W.T.reshape(NCH, 128, L).transpose(1, 0, 2).reshape(128, NCH * L)
    wt = np.ascontiguousarray(wt, dtype=np.float16)
    # b7c[p, c] = b[L-1, c*128 + p]
    b7c = np.ascontiguousarray(b[L - 1].reshape(NCH, 128).T, dtype=np.float32)

    # x: fp16, blocked transpose with (chunk, row) contiguous per partition:
    # xt[s*128+p, c*RSG+r] = x[s*RSG+r, c*128+p]
    x16 = x.astype(np.float16)
    shards = []
    for i in range(N_CORES):
        xc = x16[i * RPC : (i + 1) * RPC]                       # [RPC, D]
        xt = xc.reshape(NSG, RSG, NCH, 128).transpose(0, 3, 2, 1)
        shards.append(np.ascontiguousarray(xt).reshape(NSG * 128, NCH * RSG))

    nc = _build(cvals)

    in_maps = [{"xt": s, "wt": wt, "b7c": b7c} for s in shards]
    res = run_bass_kernel_spmd(nc, in_maps, core_ids=list(range(N_CORES)))
    LAST_RESULTS = res

    out = np.empty((B, D), dtype=np.float32)
    for i in range(N_CORES):
        yt = res.results[i]["yt"].reshape(NSG, 128, NCH, RSG)
        out[i * RPC : (i + 1) * RPC] = (
            yt.transpose(0, 3, 2, 1).reshape(RPC, D).astype(np.float32)
        )
    return out
